# revision 13
# baseline (speedup 1.0000x reference)
"""FlowNet-style correlation layer (B=4, C=128, H=W=192, k=9, stride=1) on 8 trn2 cores.

Design (per core; cores = 4 batches x 2 H-halves, SPMD):
  - Host pre-blocks x into per-patch-contiguous layout [c, blk, 128] (bf16,
    prescaled by 1/C — exact exponent shift) and pads y to h-major
    [c, 104, 200] bf16 (no duplication).
  - Residents stream into SBUF just-in-time: x in 12 row chunks, y in 13
    8-row chunks, issued from inside the row loop two rows ahead so DMA
    queue-slot waits never block row-critical engine streams.
  - For each 8x16 pixel patch (144 blocks): one PE matmul contracting
    channels: lhsT = x-patch [c, 128], rhs = strided window into y
    [c, h':16, w':24] -> psum[128, 384] ("banded all-pairs":
    psum[m, n] = sum_c x[c,pix_m] * y[c,ctx_n], n = h'*24 + w').
  - Evacuate psum -> sbuf bf16 (alternating ACT/DVE plain copies) into a
    per-block-row band tile [128, 12*384].
  - Two contiguous DMAs per block row (halves, so the tail drains early)
    write the band straight to the output tensor (4.6KB packets).
  - Host gathers each pixel's 81 useful context columns
    (col = bw*384 + (hl+di)*24 + (wl+dj), pure indexing, bit-identical)
    and reassembles [B, 81, 192, 192] f32.
"""

import numpy as np

B, C, H, W = 4, 128, 192, 192
K = 9                      # kernel_size
PAD = 4                    # displacement radius
NCORES = 8
HSH = H // 2               # 96 rows per core
YH, YW = HSH + 2 * PAD, W + 2 * PAD       # 104, 200
PH, PW = 8, 16             # patch shape (128 pixels)
CH, CW = PH + 2 * PAD, PW + 2 * PAD       # context 16 x 24
NCTX = CH * CW             # 384 band columns
NBH, NBW = HSH // PH, W // PW             # 12 x 12 = 144 blocks
NBLK = NBH * NBW
K2 = K * K                 # 81
ROWW = NBW * NCTX          # 4608 band elements per partition per block row
NYC = YH // PH             # 13 y chunks of 8 rows

_nc_cache = None


def _build_nc():
    import concourse.bacc as bacc
    import concourse.mybir as mybir
    import concourse.tile as tile

    bf16 = mybir.dt.bfloat16
    f32 = mybir.dt.float32

    nc = bacc.Bacc("TRN2", target_bir_lowering=False, debug=False)
    x_d = nc.dram_tensor("x", [C, NBLK * 128], bf16, kind="ExternalInput")
    y_d = nc.dram_tensor("y", [C, YH * YW], bf16, kind="ExternalInput")
    out_d = nc.dram_tensor("out", [NBH, 128, ROWW], bf16,
                           kind="ExternalOutput")

    with tile.TileContext(nc) as tc:
        with (
            tc.tile_pool(name="xres", bufs=1) as x_pool,
            tc.tile_pool(name="yres", bufs=1) as y_pool,
            tc.tile_pool(name="psum", bufs=8, space="PSUM") as psum_pool,
            tc.tile_pool(name="band", bufs=4) as band_pool,
        ):
            y_sb = y_pool.tile([C, YH * YW], bf16)
            y3 = y_sb[:].rearrange("c (h w) -> c h w", w=YW)
            xt = [x_pool.tile([C, NBW * 128], bf16, name=f"xt{r}")
                  for r in range(NBH)]

            def load_x(r):
                nc.sync.dma_start(
                    xt[r][:], x_d[:, r * NBW * 128:(r + 1) * NBW * 128])

            def load_y(k):
                nc.sync.dma_start(
                    y_sb[:, k * PH * YW:(k + 1) * PH * YW],
                    y_d[:, k * PH * YW:(k + 1) * PH * YW])

            # prologue: row-0 deps first, then row-1's
            load_x(0)
            load_y(0)
            load_y(1)
            load_x(1)
            load_y(2)

            for bh in range(NBH):
                band = band_pool.tile([128, ROWW], bf16)
                for bw in range(NBW):
                    lhsT = xt[bh][:, bw * 128:(bw + 1) * 128]
                    rhs = y3[:, PH * bh:PH * bh + CH, PW * bw:PW * bw + CW]
                    ps = psum_pool.tile([128, NCTX], f32)
                    nc.tensor.matmul(ps[:], lhsT, rhs, start=True, stop=True)
                    dstb = band[:, bw * NCTX:(bw + 1) * NCTX]
                    if bw % 2 == 0:
                        nc.scalar.activation(
                            dstb, ps[:], mybir.ActivationFunctionType.Copy)
                    else:
                        nc.vector.tensor_copy(dstb, ps[:])
                    if bw == 5:
                        nc.scalar.dma_start(out_d[bh, :, 0:ROWW // 2],
                                            band[:, 0:ROWW // 2])
                # second half write + next rows' loads
                nc.scalar.dma_start(out_d[bh, :, ROWW // 2:],
                                    band[:, ROWW // 2:])
                if bh + 2 < NBH:
                    load_x(bh + 2)
                if bh + 3 < NYC:
                    load_y(bh + 3)

    nc.compile()
    return nc


def _get_nc():
    global _nc_cache
    if _nc_cache is None:
        _nc_cache = _build_nc()
    return _nc_cache


def shard_inputs(x, y):
    import ml_dtypes
    xs_all = np.asarray(x, dtype=np.float32) * np.float32(1.0 / C)
    xb = xs_all.astype(ml_dtypes.bfloat16)
    yp = np.pad(np.asarray(y).astype(np.float32),
                ((0, 0), (0, 0), (PAD, PAD), (PAD, PAD))
                ).astype(ml_dtypes.bfloat16)
    in_maps = []
    for b in range(B):
        for hh in range(2):
            xs = xb[b, :, hh * HSH:(hh + 1) * HSH, :]     # [c, 96, 192]
            # pre-block: [c, bh, hl, bw, wl] -> [c, (bh bw), (hl wl)]
            xs = xs.reshape(C, NBH, PH, NBW, PW).transpose(0, 1, 3, 2, 4)
            xs = np.ascontiguousarray(xs.reshape(C, NBLK * 128))
            ys = yp[b, :, hh * HSH:hh * HSH + YH, :]      # [c, 104, 200]
            ys = np.ascontiguousarray(ys.reshape(C, YH * YW))
            in_maps.append({"x": xs, "y": ys})
    return in_maps


def _gather_cols():
    # col index into a [128, NBW*NCTX] band row for pixel m=(hl,wl) of block
    # bw, offset k=(di,dj): bw*NCTX + (hl+di)*CW + (wl+dj)
    m = np.arange(128)
    hl, wl = m // PW, m % PW
    di, dj = np.arange(K * K) // K, np.arange(K * K) % K
    pos = hl * CW + wl                                     # [128]
    q = di * CW + dj                                       # [81]
    bw = np.arange(NBW)
    return (bw[None, :, None] * NCTX
            + pos[:, None, None] + q[None, None, :])       # [128, 12, 81]


_COLS = _gather_cols().reshape(1, 128, NBW * K2)


def unshard_output(results):
    out = np.empty((B, K2, H, W), np.float32)
    for core, r in enumerate(results):
        arr = np.asarray(r["out"])                    # [12, 128, 4608] bf16
        sel = np.take_along_axis(arr, _COLS, axis=2)  # [12, 128, 12*81]
        b, hh = divmod(core, 2)
        o = sel.reshape(NBH, PH, PW, NBW, K2)         # [bh, hl, wl, bw, k]
        o = o.transpose(4, 0, 1, 3, 2).reshape(K2, HSH, W).astype(np.float32)
        out[b, :, hh * HSH:(hh + 1) * HSH, :] = o
    return out


def kernel(x, y, kernel_size, stride, _trace=False):
    assert int(kernel_size) == K and int(stride) == 1
    from concourse.bass_utils import run_bass_kernel_spmd
    nc = _get_nc()
    in_maps = shard_inputs(x, y)
    try:
        res = run_bass_kernel_spmd(nc, in_maps, list(range(NCORES)),
                                   trace=_trace)
    except Exception:
        if not _trace:
            raise
        res = run_bass_kernel_spmd(nc, in_maps, list(range(NCORES)))
    out = unshard_output(res.results)
    if _trace:
        return out, res
    return out


# revision 14
# speedup vs baseline: 1.2850x; 1.2850x over previous
"""FlowNet-style correlation layer (B=4, C=128, H=W=192, k=9, stride=1) on 8 trn2 cores.

Design (per core; cores = 4 batches x 2 H-halves, SPMD):
  - Host pre-blocks x into per-patch-contiguous layout [c, blk, 128] (bf16,
    prescaled by 1/C — exact exponent shift) and pads y to h-major
    [c, 104, 200] bf16 (no duplication).
  - Residents stream into SBUF just-in-time: x in 12 row chunks, y in 13
    8-row chunks, issued from inside the row loop two rows ahead so DMA
    queue-slot waits never block row-critical engine streams.
  - For each 8x16 pixel patch (144 blocks): one PE matmul contracting
    channels: lhsT = x-patch [c, 128], rhs = strided window into y
    [c, h':16, w':24] -> psum[128, 384] ("banded all-pairs":
    psum[m, n] = sum_c x[c,pix_m] * y[c,ctx_n], n = h'*24 + w').
  - Evacuate psum -> sbuf bf16 (alternating ACT/DVE plain copies) into a
    per-block-row band tile [128, 12*384].
  - Two contiguous DMAs per block row (halves, so the tail drains early)
    write the band straight to the output tensor (4.6KB packets).
  - Host gathers each pixel's 81 useful context columns
    (col = bw*384 + (hl+di)*24 + (wl+dj), pure indexing, bit-identical)
    and reassembles [B, 81, 192, 192] f32.
"""

import numpy as np

B, C, H, W = 4, 128, 192, 192
K = 9                      # kernel_size
PAD = 4                    # displacement radius
NCORES = 8
HSH = H // 2               # 96 rows per core
YH, YW = HSH + 2 * PAD, W + 2 * PAD       # 104, 200
PH, PW = 8, 16             # patch shape (128 pixels)
CH, CW = PH + 2 * PAD, PW + 2 * PAD       # context 16 x 24
NCTX = CH * CW             # 384 band columns
NBH, NBW = HSH // PH, W // PW             # 12 x 12 = 144 blocks
NBLK = NBH * NBW
K2 = K * K                 # 81
ROWW = NBW * NCTX          # 4608 band elements per partition per block row
NYC = YH // PH             # 13 y chunks of 8 rows

_nc_cache = None


def _build_nc():
    import concourse.bacc as bacc
    import concourse.mybir as mybir
    import concourse.tile as tile

    bf16 = mybir.dt.bfloat16
    f32 = mybir.dt.float32

    nc = bacc.Bacc("TRN2", target_bir_lowering=False, debug=False)
    x_d = nc.dram_tensor("x", [C, NBLK * 128], bf16, kind="ExternalInput")
    y_d = nc.dram_tensor("y", [C, YH * YW], bf16, kind="ExternalInput")
    out_d = nc.dram_tensor("out", [NBH, 128, ROWW], bf16,
                           kind="ExternalOutput")

    with tile.TileContext(nc) as tc:
        with (
            tc.tile_pool(name="xres", bufs=1) as x_pool,
            tc.tile_pool(name="yres", bufs=1) as y_pool,
            tc.tile_pool(name="psum", bufs=8, space="PSUM") as psum_pool,
            tc.tile_pool(name="band", bufs=4) as band_pool,
        ):
            y_sb = y_pool.tile([C, YH * YW], bf16)
            y3 = y_sb[:].rearrange("c (h w) -> c h w", w=YW)
            xt = [x_pool.tile([C, NBW * 128], bf16, name=f"xt{r}")
                  for r in range(NBH)]

            def load_x(r):
                nc.sync.dma_start(
                    xt[r][:], x_d[:, r * NBW * 128:(r + 1) * NBW * 128])

            def load_y(k):
                nc.sync.dma_start(
                    y_sb[:, k * PH * YW:(k + 1) * PH * YW],
                    y_d[:, k * PH * YW:(k + 1) * PH * YW])

            # prologue: row-0 deps first, then row-1's
            load_x(0)
            load_y(0)
            load_y(1)
            load_x(1)
            load_y(2)

            for bh in range(NBH):
                band = band_pool.tile([128, ROWW], bf16)
                for bw in range(NBW):
                    lhsT = xt[bh][:, bw * 128:(bw + 1) * 128]
                    rhs = y3[:, PH * bh:PH * bh + CH, PW * bw:PW * bw + CW]
                    ps = psum_pool.tile([128, NCTX], f32)
                    nc.tensor.matmul(ps[:], lhsT, rhs, start=True, stop=True)
                    dstb = band[:, bw * NCTX:(bw + 1) * NCTX]
                    if bw % 2 == 0:
                        nc.scalar.activation(
                            dstb, ps[:], mybir.ActivationFunctionType.Copy)
                    else:
                        nc.vector.tensor_copy(dstb, ps[:])
                    if bw == 5:
                        eng = nc.scalar if bh % 2 == 0 else nc.sync
                        eng.dma_start(out_d[bh, :, 0:ROWW // 2],
                                      band[:, 0:ROWW // 2])
                # second half write + next rows' loads
                eng = nc.scalar if bh % 2 == 0 else nc.sync
                eng.dma_start(out_d[bh, :, ROWW // 2:], band[:, ROWW // 2:])
                if bh + 2 < NBH:
                    load_x(bh + 2)
                if bh + 3 < NYC:
                    load_y(bh + 3)

    nc.compile()
    return nc


def _get_nc():
    global _nc_cache
    if _nc_cache is None:
        _nc_cache = _build_nc()
    return _nc_cache


def shard_inputs(x, y):
    import ml_dtypes
    xs_all = np.asarray(x, dtype=np.float32) * np.float32(1.0 / C)
    xb = xs_all.astype(ml_dtypes.bfloat16)
    yp = np.pad(np.asarray(y).astype(np.float32),
                ((0, 0), (0, 0), (PAD, PAD), (PAD, PAD))
                ).astype(ml_dtypes.bfloat16)
    in_maps = []
    for b in range(B):
        for hh in range(2):
            xs = xb[b, :, hh * HSH:(hh + 1) * HSH, :]     # [c, 96, 192]
            # pre-block: [c, bh, hl, bw, wl] -> [c, (bh bw), (hl wl)]
            xs = xs.reshape(C, NBH, PH, NBW, PW).transpose(0, 1, 3, 2, 4)
            xs = np.ascontiguousarray(xs.reshape(C, NBLK * 128))
            ys = yp[b, :, hh * HSH:hh * HSH + YH, :]      # [c, 104, 200]
            ys = np.ascontiguousarray(ys.reshape(C, YH * YW))
            in_maps.append({"x": xs, "y": ys})
    return in_maps


def _gather_cols():
    # col index into a [128, NBW*NCTX] band row for pixel m=(hl,wl) of block
    # bw, offset k=(di,dj): bw*NCTX + (hl+di)*CW + (wl+dj)
    m = np.arange(128)
    hl, wl = m // PW, m % PW
    di, dj = np.arange(K * K) // K, np.arange(K * K) % K
    pos = hl * CW + wl                                     # [128]
    q = di * CW + dj                                       # [81]
    bw = np.arange(NBW)
    return (bw[None, :, None] * NCTX
            + pos[:, None, None] + q[None, None, :])       # [128, 12, 81]


_COLS = _gather_cols().reshape(1, 128, NBW * K2)


def unshard_output(results):
    out = np.empty((B, K2, H, W), np.float32)
    for core, r in enumerate(results):
        arr = np.asarray(r["out"])                    # [12, 128, 4608] bf16
        sel = np.take_along_axis(arr, _COLS, axis=2)  # [12, 128, 12*81]
        b, hh = divmod(core, 2)
        o = sel.reshape(NBH, PH, PW, NBW, K2)         # [bh, hl, wl, bw, k]
        o = o.transpose(4, 0, 1, 3, 2).reshape(K2, HSH, W).astype(np.float32)
        out[b, :, hh * HSH:(hh + 1) * HSH, :] = o
    return out


def kernel(x, y, kernel_size, stride, _trace=False):
    assert int(kernel_size) == K and int(stride) == 1
    from concourse.bass_utils import run_bass_kernel_spmd
    nc = _get_nc()
    in_maps = shard_inputs(x, y)
    try:
        res = run_bass_kernel_spmd(nc, in_maps, list(range(NCORES)),
                                   trace=_trace)
    except Exception:
        if not _trace:
            raise
        res = run_bass_kernel_spmd(nc, in_maps, list(range(NCORES)))
    out = unshard_output(res.results)
    if _trace:
        return out, res
    return out


# revision 16
# speedup vs baseline: 1.3418x; 1.0442x over previous
"""FlowNet-style correlation layer (B=4, C=128, H=W=192, k=9, stride=1) on 8 trn2 cores.

Design (per core; cores = 4 batches x 2 H-halves, SPMD):
  - Host pre-blocks x into per-patch-contiguous layout [c, blk, 128] (bf16,
    prescaled by 1/C — exact exponent shift) and pads y to h-major
    [c, 104, 200] bf16 (no duplication).
  - Residents stream into SBUF just-in-time: x in 12 row chunks, y in 13
    8-row chunks, issued from inside the row loop two rows ahead so DMA
    queue-slot waits never block row-critical engine streams.
  - For each 8x16 pixel patch (144 blocks): one PE matmul contracting
    channels: lhsT = x-patch [c, 128], rhs = strided window into y
    [c, h':16, w':24] -> psum[128, 384] ("banded all-pairs":
    psum[m, n] = sum_c x[c,pix_m] * y[c,ctx_n], n = h'*24 + w').
  - Evacuate psum -> sbuf bf16 (alternating ACT/DVE plain copies) into a
    per-block-row band tile [128, 12*384].
  - Two contiguous DMAs per block row (halves, so the tail drains early)
    write the band straight to the output tensor (4.6KB packets).
  - Host gathers each pixel's 81 useful context columns
    (col = bw*384 + (hl+di)*24 + (wl+dj), pure indexing, bit-identical)
    and reassembles [B, 81, 192, 192] f32.
"""

import numpy as np

B, C, H, W = 4, 128, 192, 192
K = 9                      # kernel_size
PAD = 4                    # displacement radius
NCORES = 8
HSH = H // 2               # 96 rows per core
YH, YW = HSH + 2 * PAD, W + 2 * PAD       # 104, 200
PH, PW = 8, 16             # patch shape (128 pixels)
CH, CW = PH + 2 * PAD, PW + 2 * PAD       # context 16 x 24
NCTX = CH * CW             # 384 band columns
NBH, NBW = HSH // PH, W // PW             # 12 x 12 = 144 blocks
NBLK = NBH * NBW
K2 = K * K                 # 81
ROWW = NBW * NCTX          # 4608 band elements per partition per block row
NYC = YH // PH             # 13 y chunks of 8 rows

_nc_cache = None


def _build_nc():
    import concourse.bacc as bacc
    import concourse.mybir as mybir
    import concourse.tile as tile

    bf16 = mybir.dt.bfloat16
    f32 = mybir.dt.float32

    nc = bacc.Bacc("TRN2", target_bir_lowering=False, debug=False)
    x_d = nc.dram_tensor("x", [C, NBLK * 128], bf16, kind="ExternalInput")
    y_d = nc.dram_tensor("y", [C, YH * YW], bf16, kind="ExternalInput")
    out_d = nc.dram_tensor("out", [NBH, 128, ROWW], bf16,
                           kind="ExternalOutput")

    with tile.TileContext(nc) as tc:
        with (
            tc.tile_pool(name="xres", bufs=1) as x_pool,
            tc.tile_pool(name="yres", bufs=1) as y_pool,
            tc.tile_pool(name="psum", bufs=8, space="PSUM") as psum_pool,
            tc.tile_pool(name="band", bufs=4) as band_pool,
        ):
            y_sb = y_pool.tile([C, YH * YW], bf16)
            y3 = y_sb[:].rearrange("c (h w) -> c h w", w=YW)
            xt = [x_pool.tile([C, NBW * 128], bf16, name=f"xt{r}")
                  for r in range(NBH)]

            def load_x(r):
                nc.sync.dma_start(
                    xt[r][:], x_d[:, r * NBW * 128:(r + 1) * NBW * 128])

            def load_y(k):
                nc.sync.dma_start(
                    y_sb[:, k * PH * YW:(k + 1) * PH * YW],
                    y_d[:, k * PH * YW:(k + 1) * PH * YW])

            # prologue: row-0 deps first (y chunks are the long pole; the
            # first matmul needs only x blocks 0-1), then row-1's
            load_y(0)
            load_y(1)
            nc.sync.dma_start(xt[0][:, 0:256], x_d[:, 0:256])
            nc.sync.dma_start(xt[0][:, 256:], x_d[:, 256:NBW * 128])
            load_x(1)
            load_y(2)

            for bh in range(NBH):
                band = band_pool.tile([128, ROWW], bf16)
                for bw in range(NBW):
                    lhsT = xt[bh][:, bw * 128:(bw + 1) * 128]
                    rhs = y3[:, PH * bh:PH * bh + CH, PW * bw:PW * bw + CW]
                    ps = psum_pool.tile([128, NCTX], f32)
                    nc.tensor.matmul(ps[:], lhsT, rhs, start=True, stop=True)
                    dstb = band[:, bw * NCTX:(bw + 1) * NCTX]
                    if bw % 2 == 0:
                        nc.scalar.activation(
                            dstb, ps[:], mybir.ActivationFunctionType.Copy)
                    else:
                        nc.vector.tensor_copy(dstb, ps[:])
                    if bh == NBH - 1:
                        # last row: quarter writes so the tail drains early
                        q = ROWW // 4
                        if bw in (2, 5, 8):
                            i = bw // 3
                            nc.scalar.dma_start(
                                out_d[bh, :, i * q:(i + 1) * q],
                                band[:, i * q:(i + 1) * q])
                    elif bw == 5:
                        eng = nc.scalar if bh % 2 == 0 else nc.sync
                        eng.dma_start(out_d[bh, :, 0:ROWW // 2],
                                      band[:, 0:ROWW // 2])
                # final write + next rows' loads
                if bh == NBH - 1:
                    q = ROWW // 4
                    nc.scalar.dma_start(out_d[bh, :, 3 * q:],
                                        band[:, 3 * q:])
                else:
                    eng = nc.scalar if bh % 2 == 0 else nc.sync
                    eng.dma_start(out_d[bh, :, ROWW // 2:],
                                  band[:, ROWW // 2:])
                if bh + 2 < NBH:
                    load_x(bh + 2)
                if bh + 3 < NYC:
                    load_y(bh + 3)

    nc.compile()
    return nc


def _get_nc():
    global _nc_cache
    if _nc_cache is None:
        _nc_cache = _build_nc()
    return _nc_cache


def shard_inputs(x, y):
    import ml_dtypes
    xs_all = np.asarray(x, dtype=np.float32) * np.float32(1.0 / C)
    xb = xs_all.astype(ml_dtypes.bfloat16)
    yp = np.pad(np.asarray(y).astype(np.float32),
                ((0, 0), (0, 0), (PAD, PAD), (PAD, PAD))
                ).astype(ml_dtypes.bfloat16)
    in_maps = []
    for b in range(B):
        for hh in range(2):
            xs = xb[b, :, hh * HSH:(hh + 1) * HSH, :]     # [c, 96, 192]
            # pre-block: [c, bh, hl, bw, wl] -> [c, (bh bw), (hl wl)]
            xs = xs.reshape(C, NBH, PH, NBW, PW).transpose(0, 1, 3, 2, 4)
            xs = np.ascontiguousarray(xs.reshape(C, NBLK * 128))
            ys = yp[b, :, hh * HSH:hh * HSH + YH, :]      # [c, 104, 200]
            ys = np.ascontiguousarray(ys.reshape(C, YH * YW))
            in_maps.append({"x": xs, "y": ys})
    return in_maps


def _gather_cols():
    # col index into a [128, NBW*NCTX] band row for pixel m=(hl,wl) of block
    # bw, offset k=(di,dj): bw*NCTX + (hl+di)*CW + (wl+dj)
    m = np.arange(128)
    hl, wl = m // PW, m % PW
    di, dj = np.arange(K * K) // K, np.arange(K * K) % K
    pos = hl * CW + wl                                     # [128]
    q = di * CW + dj                                       # [81]
    bw = np.arange(NBW)
    return (bw[None, :, None] * NCTX
            + pos[:, None, None] + q[None, None, :])       # [128, 12, 81]


_COLS = _gather_cols().reshape(1, 128, NBW * K2)


def unshard_output(results):
    out = np.empty((B, K2, H, W), np.float32)
    for core, r in enumerate(results):
        arr = np.asarray(r["out"])                    # [12, 128, 4608] bf16
        sel = np.take_along_axis(arr, _COLS, axis=2)  # [12, 128, 12*81]
        b, hh = divmod(core, 2)
        o = sel.reshape(NBH, PH, PW, NBW, K2)         # [bh, hl, wl, bw, k]
        o = o.transpose(4, 0, 1, 3, 2).reshape(K2, HSH, W).astype(np.float32)
        out[b, :, hh * HSH:(hh + 1) * HSH, :] = o
    return out


def kernel(x, y, kernel_size, stride, _trace=False):
    assert int(kernel_size) == K and int(stride) == 1
    from concourse.bass_utils import run_bass_kernel_spmd
    nc = _get_nc()
    in_maps = shard_inputs(x, y)
    try:
        res = run_bass_kernel_spmd(nc, in_maps, list(range(NCORES)),
                                   trace=_trace)
    except Exception:
        if not _trace:
            raise
        res = run_bass_kernel_spmd(nc, in_maps, list(range(NCORES)))
    out = unshard_output(res.results)
    if _trace:
        return out, res
    return out
